# revision 45
# baseline (speedup 1.0000x reference)
"""PatchCore anomaly head kernel for 8x Trainium2 NeuronCores.

Math: h = relu(features @ W1 + b1); proj = h @ W2 + b2  [B,L,256]
      out[b,l] = min_m sqrt(max(|proj|^2 - 2 proj.mb_m + |mb_m|^2, 0))

Sharding: data-parallel over B (8 cores, one batch row each = 4096 rows).
Weights + memory bank replicated. Host pre-transposes everything so the
device kernel runs in the "features-on-free-dim" orientation:
  xT      [1024, 4096] per core (bf16)
  mbT     (-2*mb).T -> [256, 16384] (fp8 e4m3)
  m2T     [128,128] f32, m2T[p,t] = |mb_{t*128+p}|^2

Phase P (per 512-row chunk): bf16 MLP -> proj stored as fp8 e4m3
  [128,2,4096]; psq = pp*pp on ACT from PSUM (exact proj, bf16);
  x2 per 128-row block via ones-matmul, copied out on DVE.
Phase D (per 1024-row group g, 128 m-tiles t): pd [128,1024] f32 PSUM
  via 2 fp8 DoubleRow matmuls (K=256) from a 3-deep ring (3 x 2 banks);
  MLP/x2/transpose tiles use a separate 2-slot ring. The PE stream is
  kept dependency-clean so it ramps to its 2.4 GHz boost p-state (it
  falls back to 1.2 GHz whenever the stream stalls — the single
  biggest perf lever on this part).
  Drain pattern per tile (PATTERN): 'S' = DVE scalar_tensor_tensor
  (pd+m2) min acc straight from PSUM; 'a'/'b' = two ACT Identity+bias
  converts into a paired tmp [128,2,1024] f16, then ONE wide DVE
  tensor_tensor min [128,2048] against a dual accumulator. Each
  pair-TT is emitted one tile late so PSUM-draining ops jump ahead of
  it in the in-order DVE queue (keeps the pd ring moving).
  P chunks 2..7 are interleaved 2-per-group into D(g0..g2).
Phase F (per g): 8x PE-transpose 128-blocks of the merged acc, DVE
  min-reduce, +x2, clamp, sqrt -> outcols [128,32].

The cross term -2 x.m is the only fp8 quantity; x2 (from bf16 proj via
PSUM) and m2 (f32 host-side) stay accurate, so the fp8 noise enters a
term ~8x smaller than d^2 itself.
"""

import os
import sys

import numpy as np

if "/opt/trn_rl_repo" not in sys.path:
    sys.path.insert(0, "/opt/trn_rl_repo")

import ml_dtypes

BF16 = ml_dtypes.bfloat16
F8 = ml_dtypes.float8_e4m3fn

B, L, C = 8, 4096, 1024
D1, D2, M = 512, 256, 16384
ROWS = L  # rows per core (one batch element per core)
CHUNK = 512
N_CHUNKS = ROWS // CHUNK  # 8
N_MT = M // 128  # 128 memory-bank tiles
N_CORES = 8

USE_FP8 = True
# Row-groups of 1024 with a 3-deep pd ring measured faster than wider
# ragged groups with a 2-deep ring (ring depth buys drain decoupling).
GROUP_ROWS = [1024, 1024, 1024, 1024]
GROUP_OFF = [0, 1024, 2048, 3072]
GROUPS = len(GROUP_ROWS)

# drain engine assignment, cycled over t within each group:
#   'S' -> DVE scalar_tensor_tensor direct from PSUM (no ACT)
#   'a' -> ACT convert into tmp pair slot 0 (first half of a V-pair)
#   'b' -> ACT convert into tmp pair slot 1, then ONE DVE tensor_tensor
#          min over [128, 2048] against the dual accumulator
# acc slots are memset to +big on GPSIMD, so no ACT init tiles needed.
# len 128 ends on 'S' (no dangling 'a'); one period de-S'd to balance
# ACT (439us) vs DVE (454us) measured engine totals.
PATTERN = ("abSababS" * 16)[:128]
PATTERN = PATTERN[:64] + "abababab" + PATTERN[72:]
# group 0 only: keep ACT out of the first tiles so the tile scheduler
# cannot priority-invert the early drain behind next-chunk MLP ACTs.
PATTERN_G0 = "S" * 16 + PATTERN[16:]

LAST = {"exec_time_ns": None, "profile_json": None}

_BUILT = None


def _build():
    import concourse.bass as bass
    import concourse.tile as tile
    from concourse import bacc, mybir
    from contextlib import ExitStack

    f32 = mybir.dt.float32
    bf16 = mybir.dt.bfloat16
    f16 = mybir.dt.float16
    f8 = mybir.dt.float8e4
    mb_dt = f8 if USE_FP8 else bf16
    AF = mybir.ActivationFunctionType
    ALU = mybir.AluOpType
    AX = mybir.AxisListType
    PM = mybir.MatmulPerfMode
    ts = bass.ts

    nc = bacc.Bacc("TRN2", debug=False)

    xT = nc.declare_dram_parameter("xT", [8, 128, ROWS], bf16, False)
    w1 = nc.declare_dram_parameter("w1", [8, 128, D1], bf16, False)
    w2 = nc.declare_dram_parameter("w2", [4, 128, D2], bf16, False)
    b1t = nc.declare_dram_parameter("b1t", [128, 4], f32, False)
    b2t = nc.declare_dram_parameter("b2t", [128, 2], f32, False)
    mbt = nc.declare_dram_parameter("mbt", [2, 128, M], mb_dt, False)
    m2t = nc.declare_dram_parameter("m2t", [128, 128], f32, False)
    ident = nc.declare_dram_parameter("ident", [128, 128], f16, False)
    out = nc.declare_dram_parameter("out", [128, ROWS // 128], f32, True)

    with tile.TileContext(nc) as tc, ExitStack() as ctx:
        consts = ctx.enter_context(tc.tile_pool(name="consts", bufs=1))
        w1sb = consts.tile([128, 8, D1], bf16)
        w2sb = consts.tile([128, 4, D2], bf16)
        b1sb = consts.tile([128, 4], f32)
        b2sb = consts.tile([128, 2], f32)
        mbsb = consts.tile([128, 2, M], mb_dt)
        m2sb = consts.tile([128, 128], f32)
        idsb = consts.tile([128, 128], f16)
        onesb = consts.tile([128, 1], bf16)
        outcols = consts.tile([128, ROWS // 128], f32)
        d2cols = consts.tile([128, ROWS // 128], f32)
        x2cols = consts.tile([128, ROWS // 128], f32)
        ptile = consts.tile([128, 2, ROWS], mb_dt)

        # two DMA streams: weights/biases on the sync HWDGE queue; lead-in
        # x chunks + the 4MB memory bank on the gpsimd SWDGE queue.
        for k in range(8):
            nc.sync.dma_start(w1sb[:, k], w1[k])
        nc.sync.dma_start(b1sb[:], b1t[:])
        nc.sync.dma_start(b2sb[:], b2t[:])
        nc.sync.dma_start(m2sb[:], m2t[:])
        for j in range(4):
            nc.sync.dma_start(w2sb[:, j], w2[j])
        nc.sync.dma_start(idsb[:], ident[:])
        nc.gpsimd.memset(onesb[:], 1.0)

        xpool = ctx.enter_context(tc.tile_pool(name="xpool", bufs=2))
        hpool = ctx.enter_context(tc.tile_pool(name="hpool", bufs=2))
        qpool = ctx.enter_context(tc.tile_pool(name="qpool", bufs=2))
        accpool = ctx.enter_context(tc.tile_pool(name="accpool", bufs=2))
        tmppool = ctx.enter_context(tc.tile_pool(name="tmppool", bufs=4))
        smpool = ctx.enter_context(tc.tile_pool(name="smpool", bufs=4))

        # dedicated 3-deep ring for distance tiles (3 x 2 banks = 6 banks)
        psum_d = ctx.enter_context(tc.tile_pool(name="psumd", bufs=3, space="PSUM"))
        # small ring for MLP / x2 / transpose tiles (2 x 1 bank)
        psum_p = ctx.enter_context(tc.tile_pool(name="psump", bufs=2, space="PSUM"))

        def x_dma(ci, eng):
            xtile = xpool.tile([128, 8, CHUNK], bf16, name="xtile")
            for k in range(8):
                eng.dma_start(xtile[:, k], xT[k][:, ts(ci, CHUNK)])
            return xtile

        def p_chunk_gen(ci, xtile):
            """MLP chunk as ~52 fine-grained steps (one matmul-ish each) so
            interleaving into Phase D never bursts the in-order PE queue."""
            htile = hpool.tile([128, 4, CHUNK], bf16, name="htile")
            for j in range(4):
                ph = psum_p.tile([128, CHUNK], f32, tag="pp", name="ph")
                for k in range(8):
                    nc.tensor.matmul(
                        ph[:],
                        lhsT=w1sb[:, k, ts(j, 128)],
                        rhs=xtile[:, k],
                        start=(k == 0),
                        stop=(k == 7),
                    )
                    yield
                nc.vector.tensor_scalar(
                    htile[:, j], ph[:],
                    scalar1=b1sb[:, j : j + 1], scalar2=0.0,
                    op0=ALU.add, op1=ALU.max,
                )
                yield

            psq = qpool.tile([128, 2, CHUNK], bf16, name="psq")
            for d in range(2):
                pp = psum_p.tile([128, CHUNK], f32, tag="pp", name="pp")
                for j in range(4):
                    nc.tensor.matmul(
                        pp[:],
                        lhsT=w2sb[:, j, ts(d, 128)],
                        rhs=htile[:, j],
                        start=(j == 0),
                        stop=(j == 3),
                    )
                    yield
                nc.scalar.activation(
                    ptile[:, d, ts(ci, CHUNK)], pp[:], AF.Identity,
                    bias=b2sb[:, d : d + 1],
                )
                yield
                nc.scalar.activation(
                    psq[:, d], pp[:], AF.Square, bias=b2sb[:, d : d + 1]
                )
                yield

            for j in range(4):
                px = psum_p.tile([128, 1], f32, tag="pp", name="px")
                for d in range(2):
                    nc.tensor.matmul(
                        px[:],
                        lhsT=psq[:, d, ts(j, 128)],
                        rhs=onesb[:],
                        start=(d == 0),
                        stop=(d == 1),
                    )
                col = ci * 4 + j
                nc.vector.tensor_scalar(
                    x2cols[:, col : col + 1], px[:],
                    scalar1=0.0, scalar2=0.0, op0=ALU.add, op1=ALU.add,
                )
                yield

        # Phase P lead-in: x0 + x1 + memory bank on the gpsimd SWDGE queue.
        xt0 = x_dma(0, nc.gpsimd)
        xt1 = x_dma(1, nc.gpsimd)
        # memory bank in column pieces, in distance-consumption order;
        # piece 0 rides the lighter sync queue so it lands before the
        # first distance matmul needs it.
        for c in range(8):
            for k in range(2):
                eng = nc.sync if c == 0 else nc.gpsimd
                eng.dma_start(
                    mbsb[:, k, ts(c, M // 8)], mbt[k][:, ts(c, M // 8)]
                )
        for _ in p_chunk_gen(0, xt0):
            pass
        for _ in p_chunk_gen(1, xt1):
            pass

        # Phase F: per-row min across the 128 m-lanes, + x2, clamp, sqrt.
        # Emitted interleaved into the NEXT group's D so the in-order PE /
        # DVE queues never barrier on the acc chain draining.
        def f_block(g, j):
            ptr = psum_p.tile([128, 128], f16, tag="pp", name="ptr")
            nc.tensor.transpose(ptr[:], accs[g][:, 0, ts(j, 128)], idsb[:])
            mn = smpool.tile([128, 1], f32, name="mn")
            nc.vector.tensor_reduce(mn[:], ptr[:], axis=AX.X, op=ALU.min)
            col = GROUP_OFF[g] // 128 + j
            nc.vector.tensor_scalar(
                d2cols[:, col : col + 1],
                mn[:],
                scalar1=x2cols[:, col : col + 1],
                scalar2=0.0,
                op0=ALU.add,
                op1=ALU.max,
            )

        # ---------------- Phase D + F ----------------
        accs = {}
        pending = []
        xnext = None
        for g in range(GROUPS):
            GR = GROUP_ROWS[g]
            OFF = GROUP_OFF[g]
            # dual accumulator: slot 0 fed by S-tiles + pair-TTs, slot 1 by
            # pair-TTs; merged into slot 0 before the F phase.
            acc = accpool.tile([128, 2, GR], f16, name="acc")
            accs[g] = acc
            nc.gpsimd.memset(acc[:], 60000.0)
            pat = PATTERN_G0 if g == 0 else PATTERN
            tmp2 = None
            held = None
            for t in range(N_MT):
                pd = psum_d.tile([128, GR], f32, tag="pd", name="pd")
                for j in range(GR // 512):
                    nc.tensor.matmul(
                        pd[:, ts(j, 512)],
                        lhsT=mbsb[:, :, ts(t, 128)],
                        rhs=ptile[:, :, ts(OFF // 512 + j, 512)],
                        start=True,
                        stop=True,
                        perf_mode=PM.DoubleRow,
                    )
                kind = pat[t]
                if kind == "S":
                    nc.vector.scalar_tensor_tensor(
                        acc[:, 0], pd[:], m2sb[:, t : t + 1], acc[:, 0],
                        op0=ALU.add, op1=ALU.min,
                    )
                elif kind == "a":
                    tmp2 = tmppool.tile([128, 2, GR], f16, name="tmp2")
                    nc.scalar.activation(
                        tmp2[:, 0], pd[:], AF.Identity, bias=m2sb[:, t : t + 1]
                    )
                else:  # 'b': second half of the pair; TT is emitted one
                    # tile LATE (held) so the next tile's PSUM-draining op
                    # goes ahead of it in the in-order DVE queue.
                    nc.scalar.activation(
                        tmp2[:, 1], pd[:], AF.Identity, bias=m2sb[:, t : t + 1]
                    )
                    held = tmp2
                    tmp2 = None
                # flush the held pair-TT after this tile's PSUM op
                if held is not None and (kind != "b"):
                    nc.vector.tensor_tensor(
                        acc[:], acc[:], held[:], op=ALU.min
                    )
                    held = None

                # feed next group's MLP chunks one fine-grained step per
                # distance tile so the in-order PE queue never bursts;
                # x DMA issued 8 tiles ahead of its generator start
                if g < GROUPS - 1:
                    nxt = GROUP_OFF[g + 1] // 512
                    if t == 8:
                        xnext = x_dma(nxt, nc.sync)
                    elif t == 16:
                        pending.append(p_chunk_gen(nxt, xnext))
                    elif t == 64:
                        xnext = x_dma(nxt + 1, nc.sync)
                    elif t == 72:
                        pending.append(p_chunk_gen(nxt + 1, xnext))
                if pending:
                    try:
                        next(pending[0])
                    except StopIteration:
                        pending.pop(0)

                # previous group's acc merge at t==6, F blocks from t==8
                if g > 0 and t == 6:
                    nc.vector.tensor_tensor(
                        accs[g - 1][:, 0], accs[g - 1][:, 0],
                        accs[g - 1][:, 1], op=ALU.min,
                    )
                if g > 0 and 8 <= t < 8 + GROUP_ROWS[g - 1] // 128:
                    f_block(g - 1, t - 8)

            if held is not None:  # flush a held pair-TT at group end
                nc.vector.tensor_tensor(acc[:], acc[:], held[:], op=ALU.min)
                held = None
            if tmp2 is not None:  # flush a dangling first-half pair tile
                nc.vector.tensor_tensor(
                    acc[:, 0], acc[:, 0], tmp2[:, 0], op=ALU.min
                )
                tmp2 = None

        nc.vector.tensor_tensor(
            accs[GROUPS - 1][:, 0], accs[GROUPS - 1][:, 0],
            accs[GROUPS - 1][:, 1], op=ALU.min,
        )
        for j in range(GROUP_ROWS[GROUPS - 1] // 128):
            f_block(GROUPS - 1, j)

        nc.scalar.activation(outcols[:], d2cols[:], AF.Sqrt)
        nc.sync.dma_start(out[:], outcols[:])

    nc.compile()
    return nc


def _get_built():
    global _BUILT
    if _BUILT is None:
        _BUILT = _build()
    return _BUILT


def _prep_inputs(features, W1, b1, W2, b2, memory_bank):
    mb_np = F8 if USE_FP8 else BF16
    common = {}
    common["w1"] = np.ascontiguousarray(
        W1.astype(BF16).reshape(8, 128, D1)
    )
    common["w2"] = np.ascontiguousarray(W2.astype(BF16).reshape(4, 128, D2))
    common["b1t"] = np.ascontiguousarray(
        b1.astype(np.float32).reshape(4, 128).T
    )
    common["b2t"] = np.ascontiguousarray(
        b2.astype(np.float32).reshape(2, 128).T
    )
    mb32 = memory_bank.astype(np.float32)
    common["mbt"] = np.ascontiguousarray(
        (-2.0 * mb32).T.astype(mb_np).reshape(2, 128, M)
    )
    m2 = np.sum(mb32 * mb32, axis=1, dtype=np.float32)
    common["m2t"] = np.ascontiguousarray(m2.reshape(128, 128).T)
    common["ident"] = np.eye(128, dtype=np.float16)

    feats = features.astype(np.float32).reshape(B, L, C)
    in_maps = []
    for core in range(N_CORES):
        xTc = np.ascontiguousarray(
            feats[core].T.astype(BF16).reshape(8, 128, ROWS)
        )
        in_maps.append({**common, "xT": xTc})
    return in_maps


def kernel(features, W1, b1, W2, b2, memory_bank):
    from concourse.bass_utils import run_bass_kernel_spmd

    nc = _get_built()
    in_maps = _prep_inputs(features, W1, b1, W2, b2, memory_bank)
    res = run_bass_kernel_spmd(nc, in_maps, list(range(N_CORES)))
    LAST["exec_time_ns"] = res.exec_time_ns
    LAST["profile_json"] = res.profile_json
    out = np.empty((B, L), dtype=np.float32)
    for core in range(N_CORES):
        oc = np.asarray(res.results[core]["out"], dtype=np.float32)
        out[core] = oc.T.reshape(ROWS)
    return out
